# revision 6
# baseline (speedup 1.0000x reference)
"""ChunkLayer compaction gather for Trainium2 (8 NeuronCores, SPMD).

Problem: x [B, S, D] f32, boundaries [B, S] bool.
  num_tokens = boundaries.sum(-1); max_chunks = max(num_tokens)
  For each row, stable-compact boundary-token indices to the front
  (tail = non-boundary indices in order), take first max_chunks, and
  gather those rows of x -> [B, max_chunks, D].

Strategy: the index computation is O(B*S) on a 32KB mask -> host.
The heavy data movement (gathering ~max_chunks rows of 4KB per batch
row) runs on device via indirect DMA (SWDGE gather).

Sharding (8 cores, B=4): core c handles batch row c//2 and D-half
c%2.  Each core gathers [max_chunks, D/2] from its [S, D/2] shard:
~4.5MB read + ~4.5MB write per core, perfectly balanced.
"""

import sys

for _p in (
    "/root/.axon_site",
    "/root/.axon_site/_ro/trn_rl_repo",
    "/root/.axon_site/_ro/pypackages",
):
    if _p not in sys.path:
        sys.path.append(_p)

import numpy as np

P = 128  # SBUF partitions


def _compute_sel(boundaries: np.ndarray):
    """Replicate the reference's stable-argsort compaction exactly."""
    B, S = boundaries.shape
    num_tokens = boundaries.sum(axis=-1).astype(np.int32)
    max_chunks = int(num_tokens.max()) if B > 0 else 0
    token_idx = np.arange(S, dtype=np.int32)[None, :] + (
        (~boundaries).astype(np.int32) * S
    )
    sel = np.argsort(token_idx, axis=1, kind="stable")[:, :max_chunks].astype(np.int32)
    return sel, num_tokens, max_chunks


def _build_nc(
    S: int,
    Dh: int,
    H: int,
    T: int,
    bufs: int = 4,
    loop_iters: int | None = None,
):
    """Bass program: gather H rows (idx-selected) of a [S, Dh] f32 tensor.

    idx is provided pre-transposed as [P, T] so a single straight DMA
    loads it; column t holds the indices for output tile t.

    loop_iters: if set, wrap the body in a dynamic For_i that repeats it
    (same output each time) — used only for benchmarking.
    """
    import concourse.bass as bass
    import concourse.bacc as bacc
    import concourse.mybir as mybir
    import concourse.tile as tile

    nc = bacc.Bacc("TRN2", target_bir_lowering=False, debug=False, num_devices=8)
    xs = nc.declare_dram_parameter("xh", [S, Dh], mybir.dt.float32, isOutput=False)
    idx = nc.declare_dram_parameter("idx", [P, T], mybir.dt.int32, isOutput=False)
    out = nc.declare_dram_parameter("out", [H, Dh], mybir.dt.float32, isOutput=True)

    with tile.TileContext(nc) as tc:
        with (
            tc.tile_pool(name="idxp", bufs=1) as ip,
            tc.tile_pool(name="data", bufs=bufs) as dp,
        ):
            idx_tile = ip.tile([P, T], mybir.dt.int32)
            nc.sync.dma_start(out=idx_tile[:, :], in_=idx[:, :])

            def body():
                for t in range(T):
                    r = min(P, H - t * P)
                    data_tile = dp.tile([P, Dh], mybir.dt.float32)
                    nc.gpsimd.indirect_dma_start(
                        out=data_tile[:r, :],
                        out_offset=None,
                        in_=xs[:],
                        in_offset=bass.IndirectOffsetOnAxis(
                            ap=idx_tile[:r, t : t + 1], axis=0
                        ),
                    )
                    nc.sync.dma_start(
                        out=out[t * P : t * P + r, :], in_=data_tile[:r, :]
                    )

            if loop_iters is None:
                body()
            else:
                with tc.For_i(0, loop_iters, 1):
                    body()
    nc.finalize()
    return nc


def _run_device(x: np.ndarray, sel: np.ndarray, max_chunks: int):
    """Shard (batch row, D-half) across 8 cores, gather on device."""
    from concourse.bass_utils import run_bass_kernel_spmd

    B, S, D = x.shape
    n_split = 8 // B  # D-splits per batch row
    assert 8 % B == 0 and D % n_split == 0
    Dh = D // n_split
    H = max_chunks
    T = -(-H // P)  # ceil

    # idx laid out [P, T]: column t = sel[row, t*P:(t+1)*P] (zero-padded)
    idx_pad = np.zeros((B, T * P), dtype=np.int32)
    idx_pad[:, :H] = sel
    idx_2d = [np.ascontiguousarray(idx_pad[b].reshape(T, P).T) for b in range(B)]

    in_maps = []
    for c in range(8):
        b, h = divmod(c, n_split)
        xh = np.ascontiguousarray(x[b, :, h * Dh : (h + 1) * Dh])
        in_maps.append({"xh": xh, "idx": idx_2d[b]})

    nc = _build_nc(S, Dh, H, T)
    res = run_bass_kernel_spmd(nc, in_maps, list(range(8)))

    full = np.empty((B, H, D), dtype=np.float32)
    for c in range(8):
        b, h = divmod(c, n_split)
        full[b, :, h * Dh : (h + 1) * Dh] = res.results[c]["out"]
    return full


def kernel(x: np.ndarray, boundaries: np.ndarray):
    x = np.asarray(x, dtype=np.float32)
    boundaries = np.asarray(boundaries).astype(bool)
    B, S, D = x.shape

    sel, num_tokens, max_chunks = _compute_sel(boundaries)
    if max_chunks == 0:
        return np.zeros((B, 0, D), dtype=np.float32), num_tokens

    full = _run_device(x, sel, max_chunks)
    return full, num_tokens


# revision 8
# speedup vs baseline: 1.2810x; 1.2810x over previous
"""ChunkLayer compaction gather for Trainium2 (8 NeuronCores, SPMD).

Problem: x [B, S, D] f32, boundaries [B, S] bool.
  num_tokens = boundaries.sum(-1); max_chunks = max(num_tokens)
  For each row, stable-compact boundary-token indices to the front
  (tail = non-boundary indices in order), take first max_chunks, and
  gather those rows of x -> [B, max_chunks, D].

Strategy: the index computation is O(B*S) on a 32KB mask -> host.
The heavy data movement (gathering ~max_chunks rows of 4KB per batch
row) runs on device via indirect DMA (SWDGE gather).

Sharding (8 cores, B=4): core c handles batch row c//2 and D-half
c%2.  Each core gathers [max_chunks, D/2] from its [S, D/2] shard:
~4.5MB read + ~4.5MB write per core, perfectly balanced.
"""

import sys

for _p in (
    "/root/.axon_site",
    "/root/.axon_site/_ro/trn_rl_repo",
    "/root/.axon_site/_ro/pypackages",
):
    if _p not in sys.path:
        sys.path.append(_p)

import numpy as np

P = 128  # SBUF partitions


def _compute_sel(boundaries: np.ndarray):
    """Replicate the reference's stable-argsort compaction exactly."""
    B, S = boundaries.shape
    num_tokens = boundaries.sum(axis=-1).astype(np.int32)
    max_chunks = int(num_tokens.max()) if B > 0 else 0
    token_idx = np.arange(S, dtype=np.int32)[None, :] + (
        (~boundaries).astype(np.int32) * S
    )
    sel = np.argsort(token_idx, axis=1, kind="stable")[:, :max_chunks].astype(np.int32)
    return sel, num_tokens, max_chunks


def _build_nc(
    S: int,
    Dh: int,
    H: int,
    T: int,
    bufs: int = 8,
    loop_iters: int | None = None,
    store_split: bool = False,
):
    """Bass program: gather H rows (idx-selected) of a [S, Dh] f32 tensor.

    idx is provided pre-transposed as [P, T] so a single straight DMA
    loads it; column t holds the indices for output tile t.

    loop_iters: if set, wrap the body in a dynamic For_i that repeats it
    (same output each time) — used only for benchmarking.
    """
    import concourse.bass as bass
    import concourse.bacc as bacc
    import concourse.mybir as mybir
    import concourse.tile as tile

    nc = bacc.Bacc("TRN2", target_bir_lowering=False, debug=False, num_devices=8)
    xs = nc.declare_dram_parameter("xh", [S, Dh], mybir.dt.float32, isOutput=False)
    idx = nc.declare_dram_parameter("idx", [P, T], mybir.dt.int32, isOutput=False)
    out = nc.declare_dram_parameter("out", [H, Dh], mybir.dt.float32, isOutput=True)

    with tile.TileContext(nc) as tc:
        with (
            tc.tile_pool(name="idxp", bufs=1) as ip,
            tc.tile_pool(name="data", bufs=bufs) as dp,
        ):
            idx_tile = ip.tile([P, T], mybir.dt.int32)
            nc.sync.dma_start(out=idx_tile[:, :], in_=idx[:, :])

            def body():
                for t in range(T):
                    r = min(P, H - t * P)
                    data_tile = dp.tile([P, Dh], mybir.dt.float32)
                    nc.gpsimd.indirect_dma_start(
                        out=data_tile[:r, :],
                        out_offset=None,
                        in_=xs[:],
                        in_offset=bass.IndirectOffsetOnAxis(
                            ap=idx_tile[:r, t : t + 1], axis=0
                        ),
                    )
                    st = nc.scalar if (store_split and t % 2 == 1) else nc.sync
                    st.dma_start(
                        out=out[t * P : t * P + r, :], in_=data_tile[:r, :]
                    )

            if loop_iters is None:
                body()
            else:
                with tc.For_i(0, loop_iters, 1):
                    body()
    nc.finalize()
    return nc


def _run_device(x: np.ndarray, sel: np.ndarray, max_chunks: int):
    """Shard (batch row, D-half) across 8 cores, gather on device."""
    from concourse.bass_utils import run_bass_kernel_spmd

    B, S, D = x.shape
    n_split = 8 // B  # D-splits per batch row
    assert 8 % B == 0 and D % n_split == 0
    Dh = D // n_split
    H = max_chunks
    T = -(-H // P)  # ceil

    # idx laid out [P, T]: column t = sel[row, t*P:(t+1)*P] (zero-padded)
    idx_pad = np.zeros((B, T * P), dtype=np.int32)
    idx_pad[:, :H] = sel
    idx_2d = [np.ascontiguousarray(idx_pad[b].reshape(T, P).T) for b in range(B)]

    in_maps = []
    for c in range(8):
        b, h = divmod(c, n_split)
        xh = np.ascontiguousarray(x[b, :, h * Dh : (h + 1) * Dh])
        in_maps.append({"xh": xh, "idx": idx_2d[b]})

    nc = _build_nc(S, Dh, H, T)
    res = run_bass_kernel_spmd(nc, in_maps, list(range(8)))

    full = np.empty((B, H, D), dtype=np.float32)
    for c in range(8):
        b, h = divmod(c, n_split)
        full[b, :, h * Dh : (h + 1) * Dh] = res.results[c]["out"]
    return full


def kernel(x: np.ndarray, boundaries: np.ndarray):
    x = np.asarray(x, dtype=np.float32)
    boundaries = np.asarray(boundaries).astype(bool)
    B, S, D = x.shape

    sel, num_tokens, max_chunks = _compute_sel(boundaries)
    if max_chunks == 0:
        return np.zeros((B, 0, D), dtype=np.float32), num_tokens

    full = _run_device(x, sel, max_chunks)
    return full, num_tokens


# revision 9
# speedup vs baseline: 1.2850x; 1.0031x over previous
"""ChunkLayer compaction gather for Trainium2 (8 NeuronCores, SPMD).

Problem: x [B, S, D] f32, boundaries [B, S] bool.
  num_tokens = boundaries.sum(-1); max_chunks = max(num_tokens)
  For each row, stable-compact boundary-token indices to the front
  (tail = non-boundary indices in order), take first max_chunks, and
  gather those rows of x -> [B, max_chunks, D].

Strategy: the index computation is O(B*S) on a 32KB mask -> host.
The heavy data movement (gathering ~max_chunks rows of 4KB per batch
row) runs on device via indirect DMA (SWDGE gather), ~4.5MB read +
~4.5MB write per core, perfectly balanced across the 8 cores.

Sharding modes (B=4, 8 cores):
  dsplit: core c handles batch row c//2, D-half c%2  (2KB descriptors)
  chunk:  core c handles batch row c//2, chunk-half c%2, full D
          (4KB descriptors; x row duplicated to 2 cores)
"""

import sys

for _p in (
    "/root/.axon_site",
    "/root/.axon_site/_ro/trn_rl_repo",
    "/root/.axon_site/_ro/pypackages",
):
    if _p not in sys.path:
        sys.path.append(_p)

import numpy as np

P = 128  # SBUF partitions
MODE = "dsplit"
BUFS = 17


def _compute_sel(boundaries: np.ndarray):
    """Replicate the reference's stable-argsort compaction exactly."""
    B, S = boundaries.shape
    num_tokens = boundaries.sum(axis=-1).astype(np.int32)
    max_chunks = int(num_tokens.max()) if B > 0 else 0
    token_idx = np.arange(S, dtype=np.int32)[None, :] + (
        (~boundaries).astype(np.int32) * S
    )
    sel = np.argsort(token_idx, axis=1, kind="stable")[:, :max_chunks].astype(np.int32)
    return sel, num_tokens, max_chunks


def _build_nc(
    S: int,
    Dh: int,
    H: int,
    T: int,
    bufs: int = BUFS,
    loop_iters: int | None = None,
    store_split: bool = False,
):
    """Bass program: gather H rows (idx-selected) of a [S, Dh] f32 tensor.

    idx is provided pre-transposed as [P, T] so a single straight DMA
    loads it; column t holds the indices for output tile t.

    loop_iters: if set, wrap the body in a dynamic For_i that repeats it
    (same output each time) — used only for benchmarking.
    """
    import concourse.bass as bass
    import concourse.bacc as bacc
    import concourse.mybir as mybir
    import concourse.tile as tile

    nc = bacc.Bacc("TRN2", target_bir_lowering=False, debug=False, num_devices=8)
    xs = nc.declare_dram_parameter("xh", [S, Dh], mybir.dt.float32, isOutput=False)
    idx = nc.declare_dram_parameter("idx", [P, T], mybir.dt.int32, isOutput=False)
    out = nc.declare_dram_parameter("out", [H, Dh], mybir.dt.float32, isOutput=True)

    with tile.TileContext(nc) as tc:
        with (
            tc.tile_pool(name="idxp", bufs=1) as ip,
            tc.tile_pool(name="data", bufs=bufs) as dp,
        ):
            idx_tile = ip.tile([P, T], mybir.dt.int32)
            nc.sync.dma_start(out=idx_tile[:, :], in_=idx[:, :])

            def body():
                for t in range(T):
                    r = min(P, H - t * P)
                    data_tile = dp.tile([P, Dh], mybir.dt.float32)
                    nc.gpsimd.indirect_dma_start(
                        out=data_tile[:r, :],
                        out_offset=None,
                        in_=xs[:],
                        in_offset=bass.IndirectOffsetOnAxis(
                            ap=idx_tile[:r, t : t + 1], axis=0
                        ),
                    )
                    st = nc.scalar if (store_split and t % 2 == 1) else nc.sync
                    st.dma_start(
                        out=out[t * P : t * P + r, :], in_=data_tile[:r, :]
                    )

            if loop_iters is None:
                body()
            else:
                with tc.For_i(0, loop_iters, 1):
                    body()
    nc.finalize()
    return nc


def _pack_idx(sel_rows: np.ndarray, T: int):
    """[H] row indices -> [P, T] int32, column t = rows t*P:(t+1)*P."""
    H = sel_rows.shape[0]
    pad = np.zeros(T * P, dtype=np.int32)
    pad[:H] = sel_rows
    return np.ascontiguousarray(pad.reshape(T, P).T)


def _prep(x: np.ndarray, sel: np.ndarray, max_chunks: int, mode: str = MODE):
    """Returns (in_maps, (S, Dh, H, T), assemble_fn)."""
    B, S, D = x.shape
    H_full = max_chunks

    if mode == "dsplit":
        n_split = 8 // B
        assert 8 % B == 0 and D % n_split == 0
        Dh = D // n_split
        H = H_full
        T = -(-H // P)
        in_maps = []
        for c in range(8):
            b, h = divmod(c, n_split)
            xh = np.ascontiguousarray(x[b, :, h * Dh : (h + 1) * Dh])
            in_maps.append({"xh": xh, "idx": _pack_idx(sel[b], T)})

        def assemble(results):
            full = np.empty((B, H_full, D), dtype=np.float32)
            for c in range(8):
                b, h = divmod(c, n_split)
                full[b, :, h * Dh : (h + 1) * Dh] = results[c]["out"]
            return full

        return in_maps, (S, Dh, H, T), assemble

    if mode == "chunk":
        n_split = 8 // B
        assert n_split == 2
        H = -(-H_full // 2)  # rows per core
        T = -(-H // P)
        in_maps = []
        for c in range(8):
            b, h = divmod(c, n_split)
            part = sel[b, h * H : (h + 1) * H]
            in_maps.append({"xh": x[b], "idx": _pack_idx(part, T)})

        def assemble(results):
            full = np.empty((B, H_full, D), dtype=np.float32)
            for c in range(8):
                b, h = divmod(c, n_split)
                n = min(H, H_full - h * H)
                full[b, h * H : h * H + n, :] = results[c]["out"][:n]
            return full

        return in_maps, (S, D, H, T), assemble

    raise ValueError(mode)


def _run_device(x: np.ndarray, sel: np.ndarray, max_chunks: int, mode: str = MODE):
    from concourse.bass_utils import run_bass_kernel_spmd

    in_maps, (S, Dh, H, T), assemble = _prep(x, sel, max_chunks, mode)
    nc = _build_nc(S, Dh, H, T)
    res = run_bass_kernel_spmd(nc, in_maps, list(range(8)))
    return assemble(res.results)


def kernel(x: np.ndarray, boundaries: np.ndarray):
    x = np.asarray(x, dtype=np.float32)
    boundaries = np.asarray(boundaries).astype(bool)
    B, S, D = x.shape

    sel, num_tokens, max_chunks = _compute_sel(boundaries)
    if max_chunks == 0:
        return np.zeros((B, 0, D), dtype=np.float32), num_tokens

    full = _run_device(x, sel, max_chunks)
    return full, num_tokens


# revision 11
# speedup vs baseline: 1.4691x; 1.1433x over previous
"""ChunkLayer compaction gather for Trainium2 (8 NeuronCores, SPMD).

Problem: x [B, S, D] f32, boundaries [B, S] bool.
  num_tokens = boundaries.sum(-1); max_chunks = max(num_tokens)
  For each row, stable-compact boundary-token indices to the front
  (tail = non-boundary indices in order), take first max_chunks, and
  gather those rows of x -> [B, max_chunks, D].

Strategy: the index computation is O(B*S) on a 32KB mask -> host.
The heavy data movement (gathering ~max_chunks rows of 4KB per batch
row) runs on device via indirect DMA (SWDGE gather), ~4.5MB read +
~4.5MB write per core, perfectly balanced across the 8 cores.

Sharding modes (B=4, 8 cores):
  dsplit: core c handles batch row c//2, D-half c%2  (2KB descriptors)
  chunk:  core c handles batch row c//2, chunk-half c%2, full D
          (4KB descriptors; x row duplicated to 2 cores)
"""

import sys

for _p in (
    "/root/.axon_site",
    "/root/.axon_site/_ro/trn_rl_repo",
    "/root/.axon_site/_ro/pypackages",
):
    if _p not in sys.path:
        sys.path.append(_p)

import numpy as np

P = 128  # SBUF partitions
MODE = "dsplit"
BUFS = 17


def _compute_sel(boundaries: np.ndarray):
    """Replicate the reference's stable-argsort compaction exactly."""
    B, S = boundaries.shape
    num_tokens = boundaries.sum(axis=-1).astype(np.int32)
    max_chunks = int(num_tokens.max()) if B > 0 else 0
    token_idx = np.arange(S, dtype=np.int32)[None, :] + (
        (~boundaries).astype(np.int32) * S
    )
    sel = np.argsort(token_idx, axis=1, kind="stable")[:, :max_chunks].astype(np.int32)
    return sel, num_tokens, max_chunks


def _build_nc(
    S: int,
    Dh: int,
    H: int,
    T: int,
    bufs: int = BUFS,
    loop_iters: int | None = None,
    store_split: bool = False,
    k: int = 1,
    staggered_reset: bool = False,
    empty_body: bool = False,
):
    """Bass program: gather H rows (idx-selected) of a [S, Dh] f32 tensor.

    idx is provided pre-transposed as [P, T] so a single straight DMA
    loads it; column t holds the indices for output tile t.

    loop_iters: if set, wrap the body in a dynamic For_i that repeats it
    (same output each time) — used only for benchmarking.
    """
    import concourse.bass as bass
    import concourse.bacc as bacc
    import concourse.mybir as mybir
    import concourse.tile as tile

    nc = bacc.Bacc("TRN2", target_bir_lowering=False, debug=False, num_devices=8)
    xs = nc.declare_dram_parameter("xh", [S, Dh], mybir.dt.float32, isOutput=False)
    idx = nc.declare_dram_parameter("idx", [P, T], mybir.dt.int32, isOutput=False)
    out = nc.declare_dram_parameter("out", [H, Dh], mybir.dt.float32, isOutput=True)

    with tile.TileContext(nc) as tc:
        with (
            tc.tile_pool(name="idxp", bufs=1) as ip,
            tc.tile_pool(name="data", bufs=bufs) as dp,
        ):
            idx_tile = ip.tile([P, T], mybir.dt.int32)
            nc.sync.dma_start(out=idx_tile[:, :], in_=idx[:, :])

            def tile_group(t, kk, grp):
                """Gather+store tiles [t, t+kk) as one DMA pair (full tiles)."""
                data_tile = dp.tile([P, kk * Dh], mybir.dt.float32)
                if kk == 1:
                    gsrc = data_tile[:, :]
                    ssrc = data_tile[:, :]
                    dst = out[t * P : (t + 1) * P, :]
                else:
                    gsrc = data_tile[:].rearrange("p (k d) -> p k d", k=kk)
                    ssrc = data_tile[:].rearrange("p (k d) -> p k d", k=kk)
                    dst = out[t * P : (t + kk) * P, :].rearrange(
                        "(k p) d -> p k d", p=P
                    )
                nc.gpsimd.indirect_dma_start(
                    out=gsrc,
                    out_offset=None,
                    in_=xs[:],
                    in_offset=bass.IndirectOffsetOnAxis(
                        ap=idx_tile[:, t : t + kk], axis=0
                    ),
                )
                st = nc.scalar if (store_split and grp % 2 == 1) else nc.sync
                st.dma_start(out=dst, in_=ssrc)

            def tile_partial(t):
                r = min(P, H - t * P)
                data_tile = dp.tile([P, Dh], mybir.dt.float32, tag="data_tile")
                nc.gpsimd.indirect_dma_start(
                    out=data_tile[:r, :],
                    out_offset=None,
                    in_=xs[:],
                    in_offset=bass.IndirectOffsetOnAxis(
                        ap=idx_tile[:r, t : t + 1], axis=0
                    ),
                )
                nc.sync.dma_start(out=out[t * P : t * P + r, :], in_=data_tile[:r, :])

            def body():
                if empty_body:
                    return
                full_tiles = H // P  # tiles with all P rows valid
                t = 0
                grp = 0
                while t < full_tiles:
                    kk = min(k, full_tiles - t)
                    tile_group(t, kk, grp)
                    t += kk
                    grp += 1
                while t < T:
                    tile_partial(t)
                    t += 1

            if loop_iters is None:
                body()
            else:
                with tc.For_i(0, loop_iters, 1, staggered_reset=staggered_reset):
                    body()
    nc.finalize()
    return nc


def _pack_idx(sel_rows: np.ndarray, T: int):
    """[H] row indices -> [P, T] int32, column t = rows t*P:(t+1)*P."""
    H = sel_rows.shape[0]
    pad = np.zeros(T * P, dtype=np.int32)
    pad[:H] = sel_rows
    return np.ascontiguousarray(pad.reshape(T, P).T)


def _prep(x: np.ndarray, sel: np.ndarray, max_chunks: int, mode: str = MODE):
    """Returns (in_maps, (S, Dh, H, T), assemble_fn)."""
    B, S, D = x.shape
    H_full = max_chunks

    if mode == "dsplit":
        n_split = 8 // B
        assert 8 % B == 0 and D % n_split == 0
        Dh = D // n_split
        H = H_full
        T = -(-H // P)
        in_maps = []
        for c in range(8):
            b, h = divmod(c, n_split)
            xh = np.ascontiguousarray(x[b, :, h * Dh : (h + 1) * Dh])
            in_maps.append({"xh": xh, "idx": _pack_idx(sel[b], T)})

        def assemble(results):
            full = np.empty((B, H_full, D), dtype=np.float32)
            for c in range(8):
                b, h = divmod(c, n_split)
                full[b, :, h * Dh : (h + 1) * Dh] = results[c]["out"]
            return full

        return in_maps, (S, Dh, H, T), assemble

    if mode == "chunk":
        n_split = 8 // B
        assert n_split == 2
        H = -(-H_full // 2)  # rows per core
        T = -(-H // P)
        in_maps = []
        for c in range(8):
            b, h = divmod(c, n_split)
            part = sel[b, h * H : (h + 1) * H]
            in_maps.append({"xh": x[b], "idx": _pack_idx(part, T)})

        def assemble(results):
            full = np.empty((B, H_full, D), dtype=np.float32)
            for c in range(8):
                b, h = divmod(c, n_split)
                n = min(H, H_full - h * H)
                full[b, h * H : h * H + n, :] = results[c]["out"][:n]
            return full

        return in_maps, (S, D, H, T), assemble

    raise ValueError(mode)


def _run_device(x: np.ndarray, sel: np.ndarray, max_chunks: int, mode: str = MODE):
    from concourse.bass_utils import run_bass_kernel_spmd

    in_maps, (S, Dh, H, T), assemble = _prep(x, sel, max_chunks, mode)
    nc = _build_nc(S, Dh, H, T)
    res = run_bass_kernel_spmd(nc, in_maps, list(range(8)))
    return assemble(res.results)


def kernel(x: np.ndarray, boundaries: np.ndarray):
    x = np.asarray(x, dtype=np.float32)
    boundaries = np.asarray(boundaries).astype(bool)
    B, S, D = x.shape

    sel, num_tokens, max_chunks = _compute_sel(boundaries)
    if max_chunks == 0:
        return np.zeros((B, 0, D), dtype=np.float32), num_tokens

    full = _run_device(x, sel, max_chunks)
    return full, num_tokens


# revision 13
# speedup vs baseline: 1.5775x; 1.0737x over previous
"""ChunkLayer compaction gather for Trainium2 (8 NeuronCores, SPMD).

Problem: x [B, S, D] f32, boundaries [B, S] bool.
  num_tokens = boundaries.sum(-1); max_chunks = max(num_tokens)
  For each row, stable-compact boundary-token indices to the front
  (tail = non-boundary indices in order), take first max_chunks, and
  gather those rows of x -> [B, max_chunks, D].

Strategy: the index computation is O(B*S) on a 32KB mask -> host.
The heavy data movement (gathering ~max_chunks rows of 4KB per batch
row) runs on device via indirect DMA (SWDGE gather), ~4.5MB read +
~4.5MB write per core, perfectly balanced across the 8 cores.

Sharding modes (B=4, 8 cores):
  dsplit: core c handles batch row c//2, D-half c%2  (2KB descriptors)
  chunk:  core c handles batch row c//2, chunk-half c%2, full D
          (4KB descriptors; x row duplicated to 2 cores)
"""

import sys

for _p in (
    "/root/.axon_site",
    "/root/.axon_site/_ro/trn_rl_repo",
    "/root/.axon_site/_ro/pypackages",
):
    if _p not in sys.path:
        sys.path.append(_p)

import numpy as np

P = 128  # SBUF partitions
MODE = "dsplit"
BUFS = 17


def _compute_sel(boundaries: np.ndarray):
    """Replicate the reference's stable-argsort compaction exactly."""
    B, S = boundaries.shape
    num_tokens = boundaries.sum(axis=-1).astype(np.int32)
    max_chunks = int(num_tokens.max()) if B > 0 else 0
    token_idx = np.arange(S, dtype=np.int32)[None, :] + (
        (~boundaries).astype(np.int32) * S
    )
    sel = np.argsort(token_idx, axis=1, kind="stable")[:, :max_chunks].astype(np.int32)
    return sel, num_tokens, max_chunks


def _build_nc(
    S: int,
    Dh: int,
    H: int,
    T: int,
    bufs: int = BUFS,
    loop_iters: int | None = None,
    store_split: bool = False,
    k: int = 1,
    store_k: int = 1,
    store_eng: str = "sync",
    staggered_reset: bool = False,
    empty_body: bool = False,
):
    """Bass program: gather H rows (idx-selected) of a [S, Dh] f32 tensor.

    idx is provided pre-transposed as [P, T] so a single straight DMA
    loads it; column t holds the indices for output tile t.

    loop_iters: if set, wrap the body in a dynamic For_i that repeats it
    (same output each time) — used only for benchmarking.
    """
    import concourse.bass as bass
    import concourse.bacc as bacc
    import concourse.mybir as mybir
    import concourse.tile as tile

    nc = bacc.Bacc("TRN2", target_bir_lowering=False, debug=False, num_devices=8)
    xs = nc.declare_dram_parameter("xh", [S, Dh], mybir.dt.float32, isOutput=False)
    idx = nc.declare_dram_parameter("idx", [P, T], mybir.dt.int32, isOutput=False)
    out = nc.declare_dram_parameter("out", [H, Dh], mybir.dt.float32, isOutput=True)

    with tile.TileContext(nc) as tc:
        with (
            tc.tile_pool(name="idxp", bufs=1) as ip,
            tc.tile_pool(name="data", bufs=bufs) as dp,
        ):
            idx_tile = ip.tile([P, T], mybir.dt.int32)
            nc.sync.dma_start(out=idx_tile[:, :], in_=idx[:, :])

            def tile_group(t, kk, grp):
                """Gather+store tiles [t, t+kk) as one DMA pair (full tiles)."""
                data_tile = dp.tile([P, kk * Dh], mybir.dt.float32)
                if kk == 1:
                    gsrc = data_tile[:, :]
                    ssrc = data_tile[:, :]
                    dst = out[t * P : (t + 1) * P, :]
                else:
                    gsrc = data_tile[:].rearrange("p (k d) -> p k d", k=kk)
                    ssrc = data_tile[:].rearrange("p (k d) -> p k d", k=kk)
                    dst = out[t * P : (t + kk) * P, :].rearrange(
                        "(k p) d -> p k d", p=P
                    )
                nc.gpsimd.indirect_dma_start(
                    out=gsrc,
                    out_offset=None,
                    in_=xs[:],
                    in_offset=bass.IndirectOffsetOnAxis(
                        ap=idx_tile[:, t : t + kk], axis=0
                    ),
                )
                st = nc.scalar if (store_split and grp % 2 == 1) else nc.sync
                st.dma_start(out=dst, in_=ssrc)

            def tile_partial(t):
                r = min(P, H - t * P)
                data_tile = dp.tile([P, Dh], mybir.dt.float32, tag="data_tile")
                nc.gpsimd.indirect_dma_start(
                    out=data_tile[:r, :],
                    out_offset=None,
                    in_=xs[:],
                    in_offset=bass.IndirectOffsetOnAxis(
                        ap=idx_tile[:r, t : t + 1], axis=0
                    ),
                )
                nc.sync.dma_start(out=out[t * P : t * P + r, :], in_=data_tile[:r, :])

            def store_group(t, kk, grp):
                """kk per-tile gathers into one wide tile; one fat store."""
                data_tile = dp.tile([P, kk * Dh], mybir.dt.float32, tag="data_tile")
                for j in range(kk):
                    nc.gpsimd.indirect_dma_start(
                        out=data_tile[:, j * Dh : (j + 1) * Dh],
                        out_offset=None,
                        in_=xs[:],
                        in_offset=bass.IndirectOffsetOnAxis(
                            ap=idx_tile[:, t + j : t + j + 1], axis=0
                        ),
                    )
                st = nc.scalar if (store_split and grp % 2 == 1) else getattr(
                    nc, store_eng
                )
                if kk == 1:
                    st.dma_start(
                        out=out[t * P : (t + 1) * P, :], in_=data_tile[:, :]
                    )
                else:
                    dst = out[t * P : (t + kk) * P, :].rearrange(
                        "(k p) d -> p k d", p=P
                    )
                    src = data_tile[:].rearrange("p (k d) -> p k d", k=kk)
                    st.dma_start(out=dst, in_=src)

            def body():
                if empty_body:
                    return
                full_tiles = H // P  # tiles with all P rows valid
                t = 0
                grp = 0
                if store_k > 1:
                    while t < full_tiles:
                        kk = min(store_k, full_tiles - t)
                        store_group(t, kk, grp)
                        t += kk
                        grp += 1
                else:
                    while t < full_tiles:
                        kk = min(k, full_tiles - t)
                        tile_group(t, kk, grp)
                        t += kk
                        grp += 1
                while t < T:
                    tile_partial(t)
                    t += 1

            if loop_iters is None:
                body()
            else:
                with tc.For_i(0, loop_iters, 1, staggered_reset=staggered_reset):
                    body()
    nc.finalize()
    return nc


def _pack_idx(sel_rows: np.ndarray, T: int):
    """[H] row indices -> [P, T] int32, column t = rows t*P:(t+1)*P."""
    H = sel_rows.shape[0]
    pad = np.zeros(T * P, dtype=np.int32)
    pad[:H] = sel_rows
    return np.ascontiguousarray(pad.reshape(T, P).T)


def _prep(x: np.ndarray, sel: np.ndarray, max_chunks: int, mode: str = MODE):
    """Returns (in_maps, (S, Dh, H, T), assemble_fn)."""
    B, S, D = x.shape
    H_full = max_chunks

    if mode == "dsplit":
        n_split = 8 // B
        assert 8 % B == 0 and D % n_split == 0
        Dh = D // n_split
        H = H_full
        T = -(-H // P)
        in_maps = []
        for c in range(8):
            b, h = divmod(c, n_split)
            xh = np.ascontiguousarray(x[b, :, h * Dh : (h + 1) * Dh])
            in_maps.append({"xh": xh, "idx": _pack_idx(sel[b], T)})

        def assemble(results):
            full = np.empty((B, H_full, D), dtype=np.float32)
            for c in range(8):
                b, h = divmod(c, n_split)
                full[b, :, h * Dh : (h + 1) * Dh] = results[c]["out"]
            return full

        return in_maps, (S, Dh, H, T), assemble

    if mode == "chunk":
        n_split = 8 // B
        assert n_split == 2
        H = -(-H_full // 2)  # rows per core
        T = -(-H // P)
        in_maps = []
        for c in range(8):
            b, h = divmod(c, n_split)
            part = sel[b, h * H : (h + 1) * H]
            in_maps.append({"xh": x[b], "idx": _pack_idx(part, T)})

        def assemble(results):
            full = np.empty((B, H_full, D), dtype=np.float32)
            for c in range(8):
                b, h = divmod(c, n_split)
                n = min(H, H_full - h * H)
                full[b, h * H : h * H + n, :] = results[c]["out"][:n]
            return full

        return in_maps, (S, D, H, T), assemble

    raise ValueError(mode)


def _run_device(x: np.ndarray, sel: np.ndarray, max_chunks: int, mode: str = MODE):
    from concourse.bass_utils import run_bass_kernel_spmd

    in_maps, (S, Dh, H, T), assemble = _prep(x, sel, max_chunks, mode)
    nc = _build_nc(S, Dh, H, T)
    res = run_bass_kernel_spmd(nc, in_maps, list(range(8)))
    return assemble(res.results)


def kernel(x: np.ndarray, boundaries: np.ndarray):
    x = np.asarray(x, dtype=np.float32)
    boundaries = np.asarray(boundaries).astype(bool)
    B, S, D = x.shape

    sel, num_tokens, max_chunks = _compute_sel(boundaries)
    if max_chunks == 0:
        return np.zeros((B, 0, D), dtype=np.float32), num_tokens

    full = _run_device(x, sel, max_chunks)
    return full, num_tokens
